# revision 41
# baseline (speedup 1.0000x reference)
"""Trainium2 Bass kernel for Llama-style GQA attention (B=2,S=2048,H=4096,NH=32,NKV=8,HD=128).

Sharding: tensor-parallel over heads — core c owns Q-heads 4c..4c+3 and GQA KV-head c
(Wq/Wk/Wv column-parallel, Wo row-parallel), ReduceScatter over token rows for the
output projection. kernel(**inputs) takes full inputs, returns the full output.
"""

import math
import os
from contextlib import ExitStack

import numpy as np

B, S, H = 2, 2048, 4096
NH, NKV, HD = 32, 8, 128
THETA = 1000000.0
NCORES = 8
QH = NH // NCORES            # 4 q-heads per core
TOK = B * S                  # 4096 tokens (flattened batch*seq)
QO = QH * HD                 # 512 q-out dims per core
TT = TOK // 128              # 32 token tiles of 128
TS = TOK // 512              # 8 token slices of 512
SB = S // 512                # 4 q-slices of 512 per batch
KTB = S // 128               # 16 k-tiles of 128 per batch
F32 = None                   # set after mybir import

LAST_EXEC_NS = None
LAST_RESULT = None

_compiled = {}


def _build():
    import concourse.bass as bass
    import concourse.mybir as mybir
    import concourse.tile as tile
    from concourse import bacc

    f32 = mybir.dt.float32
    f32r = mybir.dt.float32r            # fp32 w/ 11-bit mantissa: 1 PE cyc/row
    bf = mybir.dt.bfloat16
    nc = bacc.Bacc("TRN2", target_bir_lowering=False, debug=False,
                   num_devices=NCORES)

    def inp(name, shape, dt=f32):
        return nc.dram_tensor(name, shape, dt, kind="ExternalInput").ap()

    # hidden transposed and host-pre-tiled partition-major: xTt[ti] is one
    # flat contiguous [128, H//128, 512] block (32KB per partition line) so
    # each half-slice loads as a single big-burst DMA
    xTt = inp("xTt", (TS, 128, H // 128, 512), bf)
    # weight shards host-permuted to SBUF-resident layout [p, tile, out]
    wqP = inp("wqP", (128, H // 128, QO), bf)
    wkP = inp("wkP", (128, H // 128, HD), bf)
    wvP = inp("wvP", (128, H // 128, HD), bf)
    woP = inp("woP", (128, QH, H), bf)
    bqP = inp("bqP", (128, QH))         # bq shard as [d, head]
    bkP = inp("bkP", (128, 1))
    bvP = inp("bvP", (128, 1))
    bo8 = inp("bo8", (1, H), bf)        # bo / 8
    cosT = inp("cosT", (HD, TOK), bf)
    sinT = inp("sinT", (HD, TOK), bf)
    rotM = inp("rotM", (HD, HD), bf)  # lhsT for rotate_half_interleaved
    ident = inp("ident", (128, 128), bf)
    ones = inp("ones", (128, 128), bf)
    maskT = inp("maskT", (128, 128), bf)  # triangular causal mask (keep m>=p)

    f16 = mybir.dt.float16
    out = nc.dram_tensor("out", (TOK // NCORES, H), f16, kind="ExternalOutput").ap()
    # per-t-slice partial + reduce-scatter chunk (separate tensors keep the
    # collective's dependencies scoped to one slice, so comms overlap compute)
    partials = [nc.dram_tensor(f"partial{i}", (512, H), f16, kind="Internal").ap()
                for i in range(TS)]
    rs_outs = [nc.dram_tensor(f"rs_out{i}", (512 // NCORES, H), f16,
                              kind="Internal").ap() for i in range(TS)]

    inv_sqrt_hd = 1.0 / math.sqrt(HD)

    def mm(out, lhsT, rhs, **kw):
        nc.tensor.matmul(out, lhsT, rhs, **kw)

    with tile.TileContext(nc) as tc, ExitStack() as stk:
        # ---------------- constants + persistent activations ----------------
        cpool = stk.enter_context(tc.tile_pool(name="consts", bufs=1))
        apool = stk.enter_context(tc.tile_pool(name="acts", bufs=1))
        sp0 = stk.enter_context(tc.tile_pool(name="streams", bufs=3))

        HT0 = H // 128
        HC0 = HT0 // 4
        wq_res = apool.tile([128, H // 128, QO], bf)
        wk_res = apool.tile([128, H // 128, HD], bf)
        wv_res = apool.tile([128, H // 128, HD], bf)
        wo_res = apool.tile([128, QH, H], bf)

        # critical-path-first load order: slice 0's first x chunk + the first
        # weight chunks go ahead of everything so the first matmul can start
        # after ~1.5MB of DMA instead of the full ~14MB
        WCH = 8
        x0chunks = []
        xc = sp0.tile([128, HC0, 512], bf, name="xt")
        nc.sync.dma_start(xc[:], xTt[0, :, 0:HC0, :])
        x0chunks.append(xc)
        nc.sync.dma_start(wq_res[:, 0:WCH, :], wqP[:, 0:WCH, :])
        nc.sync.dma_start(wk_res[:, 0:WCH, :], wkP[:, 0:WCH, :])
        nc.sync.dma_start(wv_res[:, 0:WCH, :], wvP[:, 0:WCH, :])
        for c in range(1, 4):
            xc = sp0.tile([128, HC0, 512], bf, name="xt")
            nc.sync.dma_start(xc[:], xTt[0, :, c * HC0:(c + 1) * HC0, :])
            x0chunks.append(xc)
        for c0 in range(WCH, H // 128, WCH):
            cs = slice(c0, c0 + WCH)
            nc.sync.dma_start(wq_res[:, cs, :], wqP[:, cs, :])
            nc.sync.dma_start(wk_res[:, cs, :], wkP[:, cs, :])
            nc.sync.dma_start(wv_res[:, cs, :], wvP[:, cs, :])

        bq_sb = cpool.tile([128, QH], f32)
        nc.sync.dma_start(bq_sb[:], bqP[:])
        bk_sb = cpool.tile([128, 1], f32)
        nc.sync.dma_start(bk_sb[:], bkP[:])
        bv_sb = cpool.tile([128, 1], f32)
        nc.sync.dma_start(bv_sb[:], bvP[:])
        rot_sb = cpool.tile([128, 128], bf)
        nc.sync.dma_start(rot_sb[:], rotM[:])
        id_sb = cpool.tile([128, 128], bf)
        nc.sync.dma_start(id_sb[:], ident[:])
        ones_sb = cpool.tile([128, 128], bf)
        nc.sync.dma_start(ones_sb[:], ones[:])
        mask_sb = cpool.tile([128, 128], bf)
        nc.sync.dma_start(mask_sb[:], maskT[:])
        cos_sb = cpool.tile([128, TOK], bf)
        nc.sync.dma_start(cos_sb[:], cosT[:])
        sin_sb = cpool.tile([128, TOK], bf)
        nc.sync.dma_start(sin_sb[:], sinT[:])
        bo8_sb = cpool.tile([1, H], bf)
        nc.sync.dma_start(bo8_sb[:], bo8[:])
        # bo/8 broadcast to all partitions, used in the O-proj PSUM drain
        bo_bc = cpool.tile([128, H], bf)
        nc.gpsimd.partition_broadcast(bo_bc[:], bo8_sb[:])

        for o in range(QH):
            nc.sync.dma_start(wo_res[:, o, :], woP[:, o, :])

        KT = apool.tile([128, TOK], bf)        # K^T (rope'd), grows causally
        Vsb = apool.tile([128, TT, 128], bf)   # V in [t mod 128, t tile, d]

        # single fused causal loop over 512-token slices; one shared PSUM tag
        sp = sp0
        tp = stk.enter_context(tc.tile_pool(name="tmps", bufs=2))
        qtp = stk.enter_context(tc.tile_pool(name="qts", bufs=2))
        vtp = stk.enter_context(tc.tile_pool(name="vts", bufs=2))
        atp = stk.enter_context(tc.tile_pool(name="attw", bufs=6))
        smp = stk.enter_context(tc.tile_pool(name="smalls", bufs=2))
        anp = stk.enter_context(tc.tile_pool(name="atn", bufs=2))
        stp = stk.enter_context(tc.tile_pool(name="ostage", bufs=2))
        dap = stk.enter_context(tc.tile_pool(name="dacc", bufs=2))
        pp = stk.enter_context(tc.tile_pool(name="ps", bufs=6, space="PSUM"))
        # separate pool for the two long-lived attention accumulators so the
        # rotating score tiles never land on their banks (which would stall
        # the next head's first matmul behind this head's softmax epilogue)
        pb = stk.enter_context(tc.tile_pool(name="psacc", bufs=2, space="PSUM"))

        def ps_tile(shape=(128, 512)):
            return pp.tile(list(shape), f32, name="ps", tag="ps")

        def ps_acc(shape=(128, 512)):
            return pb.tile(list(shape), f32, name="psa", tag="psa")

        HT = H // 128                  # 32 h tiles
        HC = HT // 4                   # 8 h tiles per load chunk
        for ti in range(TS):
            b, j = ti // SB, ti % SB
            t0 = ti * 512
            # ---- QKV projection for this token slice (accumulate over h) ----
            psq = [ps_tile() for _ in range(QH)]
            psk = ps_tile()
            psv = ps_tile()
            if ti == 0:
                xchunks = x0chunks
            else:
                xchunks = []
                for c in range(4):
                    xc = sp.tile([128, HC, 512], bf, name="xt")
                    nc.sync.dma_start(xc[:], xTt[ti, :, c * HC:(c + 1) * HC, :])
                    xchunks.append(xc)
            for hi in range(HT):
                xt = xchunks[hi // HC][:, hi % HC, :]
                st = (hi == 0)
                en = (hi == HT - 1)
                for q in range(QH):
                    mm(psq[q][:], wq_res[:, hi, q * 128:(q + 1) * 128],
                       xt, start=st, stop=en)
                mm(psk[:], wk_res[:, hi, :], xt, start=st, stop=en)
                mm(psv[:], wv_res[:, hi, :], xt, start=st, stop=en)

            # bias add (per-partition) while draining PSUM. K first — it
            # unblocks rope(K) and with it the whole attention phase; V on
            # the vector engine so the drains run two-wide.
            QTs = qtp.tile([128, QH, 512], bf, name="QTs")
            VTs = vtp.tile([128, 512], bf, name="VTs")
            nc.scalar.add(KT[:, t0:t0 + 512], psk[:], bk_sb[:, 0:1])
            nc.vector.tensor_scalar_add(VTs[:], psv[:], bv_sb[:, 0:1])
            for q in range(QH):
                nc.scalar.add(QTs[:, q, :], psq[q][:], bq_sb[:, q:q + 1])

            # rope in place on QT / KT slices
            def rope(ap_slice):
                rps = ps_acc()
                mm(rps[:], rot_sb[:], ap_slice, start=True, stop=True)
                t1 = tp.tile([128, 512], bf, name="t1")
                nc.vector.tensor_mul(t1[:], ap_slice, cos_sb[:, t0:t0 + 512])
                t2 = tp.tile([128, 512], bf, name="t2")
                nc.vector.tensor_mul(t2[:], rps[:], sin_sb[:, t0:t0 + 512])
                nc.vector.tensor_add(ap_slice, t1[:], t2[:])

            rope(KT[:, t0:t0 + 512])   # first: scores need K before all Q heads
            for q in range(QH):
                rope(QTs[:, q, :])

            # V^T -> V (PE transpose of 128x128 blocks)
            for s4 in range(4):
                g = ti * 4 + s4
                vps = pp.tile([128, 128], bf, name="vps", tag="ps")
                nc.tensor.transpose(vps[:], VTs[:, s4 * 128:(s4 + 1) * 128],
                                    id_sb[:])
                nc.scalar.copy(Vsb[:, g, :], vps[:])

            # ---- causal attention for this q slice ----
            nk = 4 * j + 4                # k tiles of 128 within batch b
            ATn = anp.tile([128, QH, 512], bf, name="ATn")
            for h in range(QH):
                at_ps = ps_acc()
                # per-k-partition partial denominators accumulate on the DVE
                # (elementwise over k tiles); one ones-matmul at the end does
                # the 128-partition reduction pre-broadcast — replaces nk PE
                # matmuls per head with a single one. bf16 partial rounding
                # averages out ~1/sqrt(128) in the final PE sum.
                acc = dap.tile([128, 512], bf, name="dacc")
                for ki in range(nk):
                    kg = b * KTB + ki
                    # diag tiles only cover q columns >= 128*a (causal)
                    a = ki - 4 * j
                    q0 = a * 128 if a > 0 else 0
                    sc_ps = ps_tile()
                    mm(sc_ps[:, q0:], KT[:, kg * 128:(kg + 1) * 128],
                       QTs[:, h, q0:], start=True, stop=True)
                    a_sb = atp.tile([128, 512], bf, name="a_sb")
                    nc.scalar.activation(a_sb[:, q0:], sc_ps[:, q0:],
                                         mybir.ActivationFunctionType.Exp,
                                         scale=inv_sqrt_hd)
                    if a >= 0:
                        nc.vector.tensor_mul(a_sb[:, q0:q0 + 128],
                                             a_sb[:, q0:q0 + 128], mask_sb[:])
                    if ki == 0:
                        nc.vector.tensor_copy(acc[:], a_sb[:])
                    else:
                        nc.vector.tensor_add(acc[:, q0:], acc[:, q0:],
                                             a_sb[:, q0:])
                    mm(at_ps[:, q0:], Vsb[:, kg, :], a_sb[:, q0:],
                       start=(ki == 0), stop=(ki == nk - 1),
                       skip_group_check=True)
                dnb_ps = ps_acc()
                mm(dnb_ps[:], ones_sb[:], acc[:], start=True, stop=True)
                rec = smp.tile([128, 512], f32, name="rec")
                nc.vector.reciprocal_approx_fast(rec[:], dnb_ps[:])
                nc.vector.tensor_mul(ATn[:, h, :], at_ps[:], rec[:])

            # ---- O-projection for this 512-token slice (row-parallel Wo) ----
            # stage f-blocks of a 128-token row group into one fp16 tile
            # (multi-KB partition lines) so stores are few big contiguous
            # DMAs; f innermost so consecutive matmuls share lhsT
            NF = H // 512
            for t4 in range(4):
                st_t = stp.tile([128, NF, 512], mybir.dt.float16,
                                name="st_t")
                ops = [ps_tile() if f < 6 else ps_acc() for f in range(NF)]
                for o in range(QH):
                    for f in range(NF):
                        f0 = f * 512
                        mm(ops[f][:], ATn[:, o, t4 * 128:(t4 + 1) * 128],
                           wo_res[:, o, f0:f0 + 512], start=(o == 0),
                           stop=(o == QH - 1))
                for f in range(NF):
                    f0 = f * 512
                    nc.vector.tensor_add(st_t[:, f, :], ops[f][:],
                                         bo_bc[:, f0:f0 + 512])
                # store on the scalar engine's DMA queue: keeps the sync
                # queue free for input loads (FIFO order would otherwise
                # stall next slice's loads behind these stores)
                nc.scalar.dma_start(
                    partials[ti][t4 * 128:(t4 + 1) * 128, :], st_t[:])

            # overlap the output reduction with the next slice's compute
            nc.gpsimd.collective_compute(
                "ReduceScatter", mybir.AluOpType.add,
                replica_groups=[list(range(NCORES))],
                ins=[partials[ti].opt()], outs=[rs_outs[ti].opt()],
            )
            # out-copies go one slice behind the RS triggers on the gpsimd
            # queue so no RS trigger ever queues behind a copy that is
            # still waiting on the previous RS's completion semaphore
            if ti > 0:
                nc.gpsimd.dma_start(out[(ti - 1) * 64:ti * 64, :],
                                    rs_outs[ti - 1][:])
        nc.gpsimd.dma_start(out[(TS - 1) * 64:TS * 64, :],
                            rs_outs[TS - 1][:])

    nc.compile()
    return nc


def _fp32r(x):
    """Round fp32 to fp32r (11-bit mantissa, RTNE, low 12 bits zero)."""
    u = np.ascontiguousarray(x, np.float32).view(np.uint32)
    lsb = (u >> 12) & 1
    out = ((u + 0x7FF + lsb) & np.uint32(0xFFFFF000)).view(np.float32)
    return out


def _host_inputs(hidden_states, position_ids, Wq, bq, Wk, bk, Wv, bv, Wo, bo):
    import ml_dtypes
    bf16 = ml_dtypes.bfloat16
    f = np.float32
    X = np.asarray(hidden_states, f).reshape(TOK, H)
    xT = np.ascontiguousarray(X.T).astype(bf16)
    # [ti, p, hi, c] = xT[hi*128+p, ti*512+c]: partition-major so each
    # slice is one flat contiguous 4MB block
    xTt = np.ascontiguousarray(
        xT.reshape(H // 128, 128, TS, 512).transpose(2, 1, 0, 3))

    pos = np.asarray(position_ids).astype(f).reshape(TOK)
    inv_freq = (1.0 / (THETA ** (np.arange(0, HD, 2, dtype=f) / HD))).astype(f)
    M = inv_freq[:, None] * pos[None, :]              # [64, TOK]
    cosT = np.repeat(np.cos(M), 2, axis=0).astype(f)  # [128, TOK]
    sinT = np.repeat(np.sin(M), 2, axis=0).astype(f)

    rotM = np.zeros((HD, HD), f)
    for i in range(HD // 2):
        rotM[2 * i + 1, 2 * i] = -1.0   # out[2i]   = -in[2i+1]
        rotM[2 * i, 2 * i + 1] = 1.0    # out[2i+1] =  in[2i]

    shared = {
        "xTt": xTt, "cosT": cosT.astype(bf16), "sinT": sinT.astype(bf16),
        "rotM": rotM.astype(bf16),
        "ident": np.eye(128, dtype=f).astype(bf16),
        "ones": np.ones((128, 128), bf16),
        "bo8": (np.asarray(bo, f) / NCORES).reshape(1, H).astype(bf16),
        "maskT": (np.arange(128)[None, :]
                  >= np.arange(128)[:, None]).astype(bf16),
    }
    Wq, Wk, Wv, Wo = (np.asarray(a, f) for a in (Wq, Wk, Wv, Wo))
    bq, bk, bv = (np.asarray(a, f) for a in (bq, bk, bv))
    in_maps = []
    for c in range(NCORES):
        m = dict(shared)
        # [p, h-tile, o] resident layout: wT[h, o] with h = ht*128 + p
        wqT = Wq[c * QO:(c + 1) * QO, :].T.reshape(H // 128, 128, QO)
        m["wqP"] = np.ascontiguousarray(wqT.transpose(1, 0, 2)).astype(bf16)
        wkT = Wk[c * HD:(c + 1) * HD, :].T.reshape(H // 128, 128, HD)
        m["wkP"] = np.ascontiguousarray(wkT.transpose(1, 0, 2)).astype(bf16)
        wvT = Wv[c * HD:(c + 1) * HD, :].T.reshape(H // 128, 128, HD)
        m["wvP"] = np.ascontiguousarray(wvT.transpose(1, 0, 2)).astype(bf16)
        woT = Wo[:, c * QO:(c + 1) * QO].T.reshape(QH, 128, H)
        m["woP"] = np.ascontiguousarray(woT.transpose(1, 0, 2)).astype(bf16)
        m["bqP"] = np.ascontiguousarray(bq[c * QO:(c + 1) * QO].reshape(QH, 128).T)
        m["bkP"] = bk[c * HD:(c + 1) * HD].reshape(128, 1).copy()
        m["bvP"] = bv[c * HD:(c + 1) * HD].reshape(128, 1).copy()
        in_maps.append(m)
    return in_maps


def kernel(hidden_states, position_ids, Wq, bq, Wk, bk, Wv, bv, Wo, bo):
    global LAST_EXEC_NS, LAST_RESULT
    from concourse.bass_utils import run_bass_kernel_spmd

    if "nc" not in _compiled:
        _compiled["nc"] = _build()
    nc = _compiled["nc"]

    in_maps = _host_inputs(hidden_states, position_ids,
                           Wq, bq, Wk, bk, Wv, bv, Wo, bo)
    trace = os.environ.get("KERNEL_TRACE", "0") == "1"
    res = run_bass_kernel_spmd(nc, in_maps, core_ids=list(range(NCORES)),
                               trace=trace)
    LAST_EXEC_NS = res.exec_time_ns
    LAST_RESULT = res
    # core c's out rows are [slice ti][64-row block c]: row ti*64+r on core c
    # holds global token ti*512 + 64*c + r
    stacked = np.stack([np.asarray(res.results[c]["out"], np.float32)
                        for c in range(NCORES)])
    full = stacked.reshape(NCORES, TS, 64, H).transpose(1, 0, 2, 3)
    return np.ascontiguousarray(full).reshape(B, S, H)



# revision 42
# speedup vs baseline: 1.0069x; 1.0069x over previous
"""Trainium2 Bass kernel for Llama-style GQA attention (B=2,S=2048,H=4096,NH=32,NKV=8,HD=128).

Sharding: tensor-parallel over heads — core c owns Q-heads 4c..4c+3 and GQA KV-head c
(Wq/Wk/Wv column-parallel, Wo row-parallel), ReduceScatter over token rows for the
output projection. kernel(**inputs) takes full inputs, returns the full output.
"""

import math
import os
from contextlib import ExitStack

import numpy as np

B, S, H = 2, 2048, 4096
NH, NKV, HD = 32, 8, 128
THETA = 1000000.0
NCORES = 8
QH = NH // NCORES            # 4 q-heads per core
TOK = B * S                  # 4096 tokens (flattened batch*seq)
QO = QH * HD                 # 512 q-out dims per core
TT = TOK // 128              # 32 token tiles of 128
TS = TOK // 512              # 8 token slices of 512
SB = S // 512                # 4 q-slices of 512 per batch
KTB = S // 128               # 16 k-tiles of 128 per batch
F32 = None                   # set after mybir import

LAST_EXEC_NS = None
LAST_RESULT = None

_compiled = {}


def _build():
    import concourse.bass as bass
    import concourse.mybir as mybir
    import concourse.tile as tile
    from concourse import bacc

    f32 = mybir.dt.float32
    f32r = mybir.dt.float32r            # fp32 w/ 11-bit mantissa: 1 PE cyc/row
    bf = mybir.dt.bfloat16
    nc = bacc.Bacc("TRN2", target_bir_lowering=False, debug=False,
                   num_devices=NCORES)

    def inp(name, shape, dt=f32):
        return nc.dram_tensor(name, shape, dt, kind="ExternalInput").ap()

    # hidden transposed and host-pre-tiled partition-major: xTt[ti] is one
    # flat contiguous [128, H//128, 512] block (32KB per partition line) so
    # each half-slice loads as a single big-burst DMA
    xTt = inp("xTt", (TS, 128, H // 128, 512), bf)
    # weight shards host-permuted to SBUF-resident layout [p, tile, out]
    wqP = inp("wqP", (128, H // 128, QO), bf)
    wkP = inp("wkP", (128, H // 128, HD), bf)
    wvP = inp("wvP", (128, H // 128, HD), bf)
    woP = inp("woP", (128, QH, H), bf)
    bqP = inp("bqP", (128, QH))         # bq shard as [d, head]
    bkP = inp("bkP", (128, 1))
    bvP = inp("bvP", (128, 1))
    bo8 = inp("bo8", (1, H), bf)        # bo / 8
    cosT = inp("cosT", (HD, TOK), bf)
    sinT = inp("sinT", (HD, TOK), bf)
    rotM = inp("rotM", (HD, HD), bf)  # lhsT for rotate_half_interleaved
    ident = inp("ident", (128, 128), bf)
    ones = inp("ones", (128, 128), bf)
    maskT = inp("maskT", (128, 128), bf)  # triangular causal mask (keep m>=p)

    f16 = mybir.dt.float16
    out = nc.dram_tensor("out", (TOK // NCORES, H), f16, kind="ExternalOutput").ap()
    # per-t-slice partial + reduce-scatter chunk (separate tensors keep the
    # collective's dependencies scoped to one slice, so comms overlap compute)
    partials = [nc.dram_tensor(f"partial{i}", (512, H), f16, kind="Internal").ap()
                for i in range(TS)]
    rs_outs = [nc.dram_tensor(f"rs_out{i}", (512 // NCORES, H), f16,
                              kind="Internal").ap() for i in range(TS)]

    inv_sqrt_hd = 1.0 / math.sqrt(HD)

    def mm(out, lhsT, rhs, **kw):
        nc.tensor.matmul(out, lhsT, rhs, **kw)

    with tile.TileContext(nc) as tc, ExitStack() as stk:
        # ---------------- constants + persistent activations ----------------
        cpool = stk.enter_context(tc.tile_pool(name="consts", bufs=1))
        apool = stk.enter_context(tc.tile_pool(name="acts", bufs=1))
        sp0 = stk.enter_context(tc.tile_pool(name="streams", bufs=3))

        HT0 = H // 128
        HC0 = HT0 // 4
        wq_res = apool.tile([128, H // 128, QO], bf)
        wk_res = apool.tile([128, H // 128, HD], bf)
        wv_res = apool.tile([128, H // 128, HD], bf)
        wo_res = apool.tile([128, QH, H], bf)

        # critical-path-first load order: slice 0's first x chunk + the first
        # weight chunks go ahead of everything so the first matmul can start
        # after ~1.5MB of DMA instead of the full ~14MB
        WCH = 8
        x0chunks = []
        xc = sp0.tile([128, HC0, 512], bf, name="xt")
        nc.sync.dma_start(xc[:], xTt[0, :, 0:HC0, :])
        x0chunks.append(xc)
        nc.sync.dma_start(wq_res[:, 0:WCH, :], wqP[:, 0:WCH, :])
        nc.sync.dma_start(wk_res[:, 0:WCH, :], wkP[:, 0:WCH, :])
        nc.sync.dma_start(wv_res[:, 0:WCH, :], wvP[:, 0:WCH, :])
        for c in range(1, 4):
            xc = sp0.tile([128, HC0, 512], bf, name="xt")
            nc.sync.dma_start(xc[:], xTt[0, :, c * HC0:(c + 1) * HC0, :])
            x0chunks.append(xc)
        for c0 in range(WCH, H // 128, WCH):
            cs = slice(c0, c0 + WCH)
            nc.sync.dma_start(wq_res[:, cs, :], wqP[:, cs, :])
            nc.sync.dma_start(wk_res[:, cs, :], wkP[:, cs, :])
            nc.sync.dma_start(wv_res[:, cs, :], wvP[:, cs, :])

        bq_sb = cpool.tile([128, QH], f32)
        nc.sync.dma_start(bq_sb[:], bqP[:])
        bk_sb = cpool.tile([128, 1], f32)
        nc.sync.dma_start(bk_sb[:], bkP[:])
        bv_sb = cpool.tile([128, 1], f32)
        nc.sync.dma_start(bv_sb[:], bvP[:])
        rot_sb = cpool.tile([128, 128], bf)
        nc.sync.dma_start(rot_sb[:], rotM[:])
        id_sb = cpool.tile([128, 128], bf)
        nc.sync.dma_start(id_sb[:], ident[:])
        ones_sb = cpool.tile([128, 128], bf)
        nc.sync.dma_start(ones_sb[:], ones[:])
        mask_sb = cpool.tile([128, 128], bf)
        nc.sync.dma_start(mask_sb[:], maskT[:])
        cos_sb = cpool.tile([128, TOK], bf)
        nc.sync.dma_start(cos_sb[:], cosT[:])
        sin_sb = cpool.tile([128, TOK], bf)
        nc.sync.dma_start(sin_sb[:], sinT[:])
        bo8_sb = cpool.tile([1, H], bf)
        nc.sync.dma_start(bo8_sb[:], bo8[:])
        # bo/8 broadcast to all partitions, used in the O-proj PSUM drain
        bo_bc = cpool.tile([128, H], bf)
        nc.gpsimd.partition_broadcast(bo_bc[:], bo8_sb[:])

        for o in range(QH):
            nc.sync.dma_start(wo_res[:, o, :], woP[:, o, :])

        KT = apool.tile([128, TOK], bf)        # K^T (rope'd), grows causally
        Vsb = apool.tile([128, TT, 128], bf)   # V in [t mod 128, t tile, d]

        # single fused causal loop over 512-token slices; one shared PSUM tag
        sp = sp0
        tp = stk.enter_context(tc.tile_pool(name="tmps", bufs=2))
        qtp = stk.enter_context(tc.tile_pool(name="qts", bufs=2))
        vtp = stk.enter_context(tc.tile_pool(name="vts", bufs=2))
        atp = stk.enter_context(tc.tile_pool(name="attw", bufs=6))
        smp = stk.enter_context(tc.tile_pool(name="smalls", bufs=2))
        anp = stk.enter_context(tc.tile_pool(name="atn", bufs=2))
        stp = stk.enter_context(tc.tile_pool(name="ostage", bufs=2))
        dap = stk.enter_context(tc.tile_pool(name="dacc", bufs=2))
        pp = stk.enter_context(tc.tile_pool(name="ps", bufs=6, space="PSUM"))
        # separate pool for the two long-lived attention accumulators so the
        # rotating score tiles never land on their banks (which would stall
        # the next head's first matmul behind this head's softmax epilogue)
        pb = stk.enter_context(tc.tile_pool(name="psacc", bufs=2, space="PSUM"))

        def ps_tile(shape=(128, 512)):
            return pp.tile(list(shape), f32, name="ps", tag="ps")

        def ps_acc(shape=(128, 512)):
            return pb.tile(list(shape), f32, name="psa", tag="psa")

        HT = H // 128                  # 32 h tiles
        HC = HT // 4                   # 8 h tiles per load chunk
        for ti in range(TS):
            b, j = ti // SB, ti % SB
            t0 = ti * 512
            # ---- QKV projection for this token slice (accumulate over h) ----
            psq = [ps_tile() for _ in range(QH)]
            psk = ps_tile()
            psv = ps_tile()
            if ti == 0:
                xchunks = x0chunks
            else:
                xchunks = []
                for c in range(4):
                    xc = sp.tile([128, HC, 512], bf, name="xt")
                    nc.sync.dma_start(xc[:], xTt[ti, :, c * HC:(c + 1) * HC, :])
                    xchunks.append(xc)
            for hi in range(HT):
                xt = xchunks[hi // HC][:, hi % HC, :]
                st = (hi == 0)
                en = (hi == HT - 1)
                for q in range(QH):
                    mm(psq[q][:], wq_res[:, hi, q * 128:(q + 1) * 128],
                       xt, start=st, stop=en)
                mm(psk[:], wk_res[:, hi, :], xt, start=st, stop=en)
                mm(psv[:], wv_res[:, hi, :], xt, start=st, stop=en)

            # bias add (per-partition) while draining PSUM. K first — it
            # unblocks rope(K) and with it the whole attention phase; V on
            # the vector engine so the drains run two-wide.
            QTs = qtp.tile([128, QH, 512], bf, name="QTs")
            VTs = vtp.tile([128, 512], bf, name="VTs")
            nc.scalar.add(KT[:, t0:t0 + 512], psk[:], bk_sb[:, 0:1])
            nc.vector.tensor_scalar_add(VTs[:], psv[:], bv_sb[:, 0:1])
            for q in range(QH):
                nc.scalar.add(QTs[:, q, :], psq[q][:], bq_sb[:, q:q + 1])

            # rope in place on QT / KT slices
            def rope(ap_slice):
                rps = ps_tile()
                mm(rps[:], rot_sb[:], ap_slice, start=True, stop=True)
                t1 = tp.tile([128, 512], bf, name="t1")
                nc.vector.tensor_mul(t1[:], ap_slice, cos_sb[:, t0:t0 + 512])
                t2 = tp.tile([128, 512], bf, name="t2")
                nc.vector.tensor_mul(t2[:], rps[:], sin_sb[:, t0:t0 + 512])
                nc.vector.tensor_add(ap_slice, t1[:], t2[:])

            rope(KT[:, t0:t0 + 512])   # first: scores need K before all Q heads
            for q in range(QH):
                rope(QTs[:, q, :])

            # V^T -> V (PE transpose of 128x128 blocks)
            for s4 in range(4):
                g = ti * 4 + s4
                vps = pp.tile([128, 128], bf, name="vps", tag="ps")
                nc.tensor.transpose(vps[:], VTs[:, s4 * 128:(s4 + 1) * 128],
                                    id_sb[:])
                nc.scalar.copy(Vsb[:, g, :], vps[:])

            # ---- causal attention for this q slice ----
            nk = 4 * j + 4                # k tiles of 128 within batch b
            ATn = anp.tile([128, QH, 512], bf, name="ATn")
            for h in range(QH):
                at_ps = ps_acc()
                # per-k-partition partial denominators accumulate on the DVE
                # (elementwise over k tiles); one ones-matmul at the end does
                # the 128-partition reduction pre-broadcast — replaces nk PE
                # matmuls per head with a single one. bf16 partial rounding
                # averages out ~1/sqrt(128) in the final PE sum.
                acc = dap.tile([128, 512], bf, name="dacc")
                for ki in range(nk):
                    kg = b * KTB + ki
                    # diag tiles only cover q columns >= 128*a (causal)
                    a = ki - 4 * j
                    q0 = a * 128 if a > 0 else 0
                    sc_ps = ps_tile()
                    mm(sc_ps[:, q0:], KT[:, kg * 128:(kg + 1) * 128],
                       QTs[:, h, q0:], start=True, stop=True)
                    a_sb = atp.tile([128, 512], bf, name="a_sb")
                    nc.scalar.activation(a_sb[:, q0:], sc_ps[:, q0:],
                                         mybir.ActivationFunctionType.Exp,
                                         scale=inv_sqrt_hd)
                    if a >= 0:
                        nc.vector.tensor_mul(a_sb[:, q0:q0 + 128],
                                             a_sb[:, q0:q0 + 128], mask_sb[:])
                    if ki == 0:
                        nc.vector.tensor_copy(acc[:], a_sb[:])
                    else:
                        nc.vector.tensor_add(acc[:, q0:], acc[:, q0:],
                                             a_sb[:, q0:])
                    mm(at_ps[:, q0:], Vsb[:, kg, :], a_sb[:, q0:],
                       start=(ki == 0), stop=(ki == nk - 1),
                       skip_group_check=True)
                dnb_ps = ps_acc()
                mm(dnb_ps[:], ones_sb[:], acc[:], start=True, stop=True)
                rec = smp.tile([128, 512], f32, name="rec")
                nc.vector.reciprocal_approx_fast(rec[:], dnb_ps[:])
                nc.vector.tensor_mul(ATn[:, h, :], at_ps[:], rec[:])

            # ---- O-projection for this 512-token slice (row-parallel Wo) ----
            # stage f-blocks of a 128-token row group into one fp16 tile
            # (multi-KB partition lines) so stores are few big contiguous
            # DMAs; f innermost so consecutive matmuls share lhsT
            NF = H // 512
            for t4 in range(4):
                st_t = stp.tile([128, NF, 512], mybir.dt.float16,
                                name="st_t")
                ops = [ps_tile() if f < 6 else ps_acc() for f in range(NF)]
                for o in range(QH):
                    for f in range(NF):
                        f0 = f * 512
                        mm(ops[f][:], ATn[:, o, t4 * 128:(t4 + 1) * 128],
                           wo_res[:, o, f0:f0 + 512], start=(o == 0),
                           stop=(o == QH - 1))
                for f in range(NF):
                    f0 = f * 512
                    nc.vector.tensor_add(st_t[:, f, :], ops[f][:],
                                         bo_bc[:, f0:f0 + 512])
                # store on the scalar engine's DMA queue: keeps the sync
                # queue free for input loads (FIFO order would otherwise
                # stall next slice's loads behind these stores)
                nc.scalar.dma_start(
                    partials[ti][t4 * 128:(t4 + 1) * 128, :], st_t[:])

            # overlap the output reduction with the next slice's compute
            nc.gpsimd.collective_compute(
                "ReduceScatter", mybir.AluOpType.add,
                replica_groups=[list(range(NCORES))],
                ins=[partials[ti].opt()], outs=[rs_outs[ti].opt()],
            )
            # out-copies go one slice behind the RS triggers on the gpsimd
            # queue so no RS trigger ever queues behind a copy that is
            # still waiting on the previous RS's completion semaphore
            if ti > 0:
                nc.gpsimd.dma_start(out[(ti - 1) * 64:ti * 64, :],
                                    rs_outs[ti - 1][:])
        nc.gpsimd.dma_start(out[(TS - 1) * 64:TS * 64, :],
                            rs_outs[TS - 1][:])

    nc.compile()
    return nc


def _fp32r(x):
    """Round fp32 to fp32r (11-bit mantissa, RTNE, low 12 bits zero)."""
    u = np.ascontiguousarray(x, np.float32).view(np.uint32)
    lsb = (u >> 12) & 1
    out = ((u + 0x7FF + lsb) & np.uint32(0xFFFFF000)).view(np.float32)
    return out


def _host_inputs(hidden_states, position_ids, Wq, bq, Wk, bk, Wv, bv, Wo, bo):
    import ml_dtypes
    bf16 = ml_dtypes.bfloat16
    f = np.float32
    X = np.asarray(hidden_states, f).reshape(TOK, H)
    xT = np.ascontiguousarray(X.T).astype(bf16)
    # [ti, p, hi, c] = xT[hi*128+p, ti*512+c]: partition-major so each
    # slice is one flat contiguous 4MB block
    xTt = np.ascontiguousarray(
        xT.reshape(H // 128, 128, TS, 512).transpose(2, 1, 0, 3))

    pos = np.asarray(position_ids).astype(f).reshape(TOK)
    inv_freq = (1.0 / (THETA ** (np.arange(0, HD, 2, dtype=f) / HD))).astype(f)
    M = inv_freq[:, None] * pos[None, :]              # [64, TOK]
    cosT = np.repeat(np.cos(M), 2, axis=0).astype(f)  # [128, TOK]
    sinT = np.repeat(np.sin(M), 2, axis=0).astype(f)

    rotM = np.zeros((HD, HD), f)
    for i in range(HD // 2):
        rotM[2 * i + 1, 2 * i] = -1.0   # out[2i]   = -in[2i+1]
        rotM[2 * i, 2 * i + 1] = 1.0    # out[2i+1] =  in[2i]

    shared = {
        "xTt": xTt, "cosT": cosT.astype(bf16), "sinT": sinT.astype(bf16),
        "rotM": rotM.astype(bf16),
        "ident": np.eye(128, dtype=f).astype(bf16),
        "ones": np.ones((128, 128), bf16),
        "bo8": (np.asarray(bo, f) / NCORES).reshape(1, H).astype(bf16),
        "maskT": (np.arange(128)[None, :]
                  >= np.arange(128)[:, None]).astype(bf16),
    }
    Wq, Wk, Wv, Wo = (np.asarray(a, f) for a in (Wq, Wk, Wv, Wo))
    bq, bk, bv = (np.asarray(a, f) for a in (bq, bk, bv))
    in_maps = []
    for c in range(NCORES):
        m = dict(shared)
        # [p, h-tile, o] resident layout: wT[h, o] with h = ht*128 + p
        wqT = Wq[c * QO:(c + 1) * QO, :].T.reshape(H // 128, 128, QO)
        m["wqP"] = np.ascontiguousarray(wqT.transpose(1, 0, 2)).astype(bf16)
        wkT = Wk[c * HD:(c + 1) * HD, :].T.reshape(H // 128, 128, HD)
        m["wkP"] = np.ascontiguousarray(wkT.transpose(1, 0, 2)).astype(bf16)
        wvT = Wv[c * HD:(c + 1) * HD, :].T.reshape(H // 128, 128, HD)
        m["wvP"] = np.ascontiguousarray(wvT.transpose(1, 0, 2)).astype(bf16)
        woT = Wo[:, c * QO:(c + 1) * QO].T.reshape(QH, 128, H)
        m["woP"] = np.ascontiguousarray(woT.transpose(1, 0, 2)).astype(bf16)
        m["bqP"] = np.ascontiguousarray(bq[c * QO:(c + 1) * QO].reshape(QH, 128).T)
        m["bkP"] = bk[c * HD:(c + 1) * HD].reshape(128, 1).copy()
        m["bvP"] = bv[c * HD:(c + 1) * HD].reshape(128, 1).copy()
        in_maps.append(m)
    return in_maps


def kernel(hidden_states, position_ids, Wq, bq, Wk, bk, Wv, bv, Wo, bo):
    global LAST_EXEC_NS, LAST_RESULT
    from concourse.bass_utils import run_bass_kernel_spmd

    if "nc" not in _compiled:
        _compiled["nc"] = _build()
    nc = _compiled["nc"]

    in_maps = _host_inputs(hidden_states, position_ids,
                           Wq, bq, Wk, bk, Wv, bv, Wo, bo)
    trace = os.environ.get("KERNEL_TRACE", "0") == "1"
    res = run_bass_kernel_spmd(nc, in_maps, core_ids=list(range(NCORES)),
                               trace=trace)
    LAST_EXEC_NS = res.exec_time_ns
    LAST_RESULT = res
    # core c's out rows are [slice ti][64-row block c]: row ti*64+r on core c
    # holds global token ti*512 + 64*c + r
    stacked = np.stack([np.asarray(res.results[c]["out"], np.float32)
                        for c in range(NCORES)])
    full = stacked.reshape(NCORES, TS, 64, H).transpose(1, 0, 2, 3)
    return np.ascontiguousarray(full).reshape(B, S, H)

